# revision 45
# baseline (speedup 1.0000x reference)
"""HGRN BitAttention Trainium2 kernel (8-core SPMD, token-sharded).

Sharding: core c handles batch c//2, sequence half c%2 (1024 tokens).
The HGRN recurrence carry crosses the half boundary via small pair
AllReduces (4 chunks, issued early so the latency hides under the
g-projection); masks make the program uniform (SPMD).

BitLinear trick: activations quantize to integers in [-127,127] and
weights to {-1,0,1} - both exact in bf16 - so all four projections are
exact-integer bf16 matmuls with fp32 PSUM accumulation; per-token /
per-weight scales are applied outside the matmuls.

Layout: everything except the final output projection result is
feature-major [feature, token].  The gate chain is algebraically
reduced so that per-token normalizers cancel before rounding:
  o_partial = g*(1/s_x)(1/ws_g)*gw * h*sigmoid(h)
  oq        = round(o_partial * 127/max_f|o_partial|)
  out       = (oq @ WoT) * SC2,  SC2 = rstd_o*rstd_g*mxp*rwso/127
Per-token sumsq reductions run on the PE (ones-column matmul), the
per-token max runs as a DVE partition-halving tree.  The o-projection
takes feature-major oq tiles as the stationary operand, so its output
is token-major with no explicit transposes anywhere except the initial
x quantization (DMA xbar).
"""

import numpy as np
import ml_dtypes

import concourse.bass as bass
import concourse.bass_isa as bass_isa
import concourse.bacc as bacc
import concourse.mybir as mybir
import concourse.tile as tile
from concourse.bass_utils import run_bass_kernel_spmd

F32 = mybir.dt.float32
BF16 = mybir.dt.bfloat16
FP16 = mybir.dt.float16
I32 = mybir.dt.int32
AF = mybir.ActivationFunctionType
OP = mybir.AluOpType

B, L, D = 4, 2048, 2048
NCORES = 8
TPC = L // 2          # tokens per core = 1024
NTT = TPC // 128      # 8 token tiles per core
KT = D // 128         # 16 k tiles
MT = D // 128         # 16 m tiles
CCH = 4               # carry-exchange chunks (4 m-tiles each)
EPS = 1e-5


def build_nc(rwsi, rwsf, rwsg, rwso):
    nc = bacc.Bacc("TRN2", target_bir_lowering=False, debug=False,
                   num_devices=NCORES)

    x_d = nc.dram_tensor("x", [TPC, D], F32, kind="ExternalInput")
    wit_d = nc.dram_tensor("wit", [MT, 128, KT, 128], BF16, kind="ExternalInput")
    wft_d = nc.dram_tensor("wft", [MT, 128, KT, 128], BF16, kind="ExternalInput")
    wgt_d = nc.dram_tensor("wgt", [MT, 128, KT, 128], BF16, kind="ExternalInput")
    wot_d = nc.dram_tensor("wot", [4, KT, 128, 512], BF16, kind="ExternalInput")
    gwf_d = nc.dram_tensor("gwf", [128, MT], F32, kind="ExternalInput")
    me_d = nc.dram_tensor("mask_even", [128, 1], F32, kind="ExternalInput")
    mo_d = nc.dram_tensor("mask_odd", [128, 1], F32, kind="ExternalInput")
    out_d = nc.dram_tensor("out", [TPC, D], F32, kind="ExternalOutput")

    with tile.TileContext(nc) as tc:
        with (
            tc.tile_pool(name="const", bufs=1) as cp,
            tc.tile_pool(name="hp", bufs=1) as hp,
            tc.tile_pool(name="dram", bufs=1, space="DRAM") as dram,
        ):
            # ---- constants ----
            me = cp.tile([128, 1], F32)
            nc.sync.dma_start(me[:], me_d.ap())
            mo = cp.tile([128, 1], F32)
            nc.sync.dma_start(mo[:], mo_d.ap())
            gwf = cp.tile([128, MT], F32)
            nc.sync.dma_start(gwf[:], gwf_d.ap())
            epsb = cp.tile([128, 1], F32)
            nc.vector.memset(epsb[:], EPS)
            zeros = cp.tile([128, TPC], F32)
            nc.vector.memset(zeros[:], 0.0)
            ones1 = cp.tile([1, 128], F32)
            nc.vector.memset(ones1[:], 1.0)
            onescol = cp.tile([128, 1], BF16)
            nc.vector.memset(onescol[:], 1.0)

            srec = cp.tile([128, NTT], F32)     # (1/s_x) per token
            S = cp.tile([128, TPC], F32)        # (1/s_x) feature-major bcast
            bnd = cp.tile([128, MT], F32)
            bnd2 = cp.tile([128, MT], F32)
            carried = cp.tile([128, MT], F32)
            sc2col = cp.tile([128, NTT], F32)
            srow = cp.tile([1, TPC], F32)
            rA = cp.tile([1, TPC], F32)
            rB = cp.tile([1, TPC], F32)
            rC = cp.tile([1, TPC], F32)

            hs = [None] * MT

            xqp_ctx = tc.tile_pool(name="xq", bufs=1)
            xqp = xqp_ctx.__enter__()
            xqT = xqp.tile([128, KT * TPC], BF16)
            xqT3 = xqT[:].rearrange("p (k t) -> p k t", k=KT)

            fcp_ctx = tc.tile_pool(name="fcp", bufs=1)
            fcp = fcp_ctx.__enter__()
            fcs = [None] * MT

            # ================= Phase X: quantize x (token-major stats) ========
            with (
                tc.tile_pool(name="xin", bufs=2) as xin,
                tc.tile_pool(name="xw", bufs=2) as xw,
            ):
                xts = {}

                def load_x(tt):
                    if tt >= NTT:
                        return
                    xt = xin.tile([128, D], F32, name="xt")
                    nc.sync.dma_start(xt[:],
                                      x_d.ap()[tt * 128:(tt + 1) * 128, :])
                    xts[tt] = xt

                qbs = {}

                def transpose(tt):
                    if tt < 0:
                        return
                    nc.scalar.dma_start_transpose(
                        xqT3[:, :, tt * 128:(tt + 1) * 128], qbs.pop(tt)[:])

                load_x(0)
                load_x(1)
                for tt in range(NTT):
                    load_x(tt + 2)
                    transpose(tt - 1)
                    xt = xts.pop(tt)
                    scr = xw.tile([128, D], F32, bufs=1)
                    ssum = xw.tile([128, 1], F32)
                    nc.scalar.activation(scr[:], xt[:], AF.Square,
                                         accum_out=ssum[:])
                    std = xw.tile([128, 1], F32)
                    nc.scalar.activation(std[:], ssum[:], AF.Sqrt,
                                         bias=epsb[:], scale=1.0 / D)
                    rstd = xw.tile([128, 1], F32)
                    nc.vector.reciprocal(rstd[:], std[:])
                    mx = xw.tile([128, 1], F32)
                    nc.vector.tensor_reduce(mx[:], xt[:], mybir.AxisListType.X,
                                            OP.max, apply_absolute_value=True)
                    a = xw.tile([128, 1], F32)
                    nc.vector.tensor_tensor(a[:], rstd[:], mx[:], OP.mult)
                    nc.vector.tensor_scalar_max(a[:], a[:], EPS)
                    nc.vector.tensor_scalar_mul(srec[:, tt:tt + 1], a[:],
                                                1.0 / 127.0)
                    r1 = xw.tile([128, 1], F32)
                    nc.vector.reciprocal(r1[:], a[:])
                    qsc = xw.tile([128, 1], F32)
                    nc.vector.tensor_tensor(qsc[:], r1[:], rstd[:], OP.mult)
                    nc.vector.tensor_scalar_mul(qsc[:], qsc[:], 127.0)
                    qi = xw.tile([128, D], I32)
                    nc.vector.tensor_scalar_mul(qi[:], xt[:], qsc[:])
                    qb = xw.tile([128, D], BF16)
                    nc.scalar.copy(qb[:], qi[:])
                    qbs[tt] = qb

                    # stage + broadcast the per-token scale for this half
                    if tt % 4 == 3:
                        c = tt // 4
                        srd = dram.tile([1, 512], F32, name=f"srd_{c}")
                        nc.sync.dma_start(
                            srd[:].rearrange("o (t p) -> (o p) t", p=128),
                            srec[:, c * 4:(c + 1) * 4])
                        nc.sync.dma_start(srow[:, c * 512:(c + 1) * 512],
                                          srd[:])
                        if c == 0:
                            nc.gpsimd.partition_broadcast(S[:, 0:512],
                                                          srow[:, 0:512])
                transpose(NTT - 1)

            # ====== Phase P: i/f projections + scans (feature-major) ======
            cin = [None] * CCH
            cout = [None] * CCH
            with (
                tc.tile_pool(name="wif", bufs=2) as wif,
                tc.tile_pool(name="psp", bufs=2, space="PSUM") as psp,
            ):
                # ====== Phase P: i/f projections + scans (feature-major) =====
                # two passes over token halves; pass 1 starts as soon as the
                # first half of x is quantized; scans chain across the halves
                wtiles = {}
                gtiles = {}

                def load_if(j):
                    if j >= 2 * MT:
                        return
                    # pass-1 weights ride the gpsimd queue (idle during the
                    # x-quantization); later ones the sync queue (drained)
                    eng = nc.gpsimd if j <= MT else nc.sync
                    mm = j % MT
                    wi = wif.tile([128, KT * 128], BF16, name="wi_m")
                    eng.dma_start(
                        wi[:], wit_d.ap()[mm].rearrange("p k c -> p (k c)"))
                    wf = wif.tile([128, KT * 128], BF16, name="wf_m")
                    eng.dma_start(
                        wf[:], wft_d.ap()[mm].rearrange("p k c -> p (k c)"))
                    wtiles[j] = (wi, wf)

                def load_g(mm):
                    if mm >= MT:
                        return
                    wg = wif.tile([128, KT * 128], BF16, name="wf_m")
                    nc.sync.dma_start(
                        wg[:], wgt_d.ap()[mm].rearrange("p k c -> p (k c)"))
                    gtiles[mm] = wg

                with tc.tile_pool(name="pw", bufs=2) as pw:
                    load_if(0)
                    for half in range(2):
                        t0 = half * 512
                        if half == 1:
                            nc.gpsimd.partition_broadcast(S[:, 512:TPC],
                                                          srow[:, 512:TPC])
                        for m in range(MT):
                            j = half * MT + m
                            load_if(j + 1)
                            wi_m, wf_m = wtiles.pop(j)
                            psi = psp.tile([128, 512], F32, name="psi")
                            psf = psp.tile([128, 512], F32, name="psf")
                            for k in range(KT):
                                li = wi_m[:, k * 128:(k + 1) * 128]
                                lf = wf_m[:, k * 128:(k + 1) * 128]
                                st, sp = (k == 0), (k == KT - 1)
                                nc.tensor.matmul(psi[:], li,
                                                 xqT3[:, k, t0:t0 + 512],
                                                 start=st, stop=sp)
                                nc.tensor.matmul(psf[:], lf,
                                                 xqT3[:, k, t0:t0 + 512],
                                                 start=st, stop=sp)
                            tmpf = pw.tile([128, 512], F32, bufs=1)
                            nc.vector.tensor_tensor(tmpf[:], psf[:],
                                                    S[:, t0:t0 + 512], OP.mult)
                            F = pw.tile([128, 512], F32, bufs=1)
                            nc.scalar.activation(F[:], tmpf[:], AF.Sigmoid,
                                                 scale=rwsf)
                            G = pw.tile([128, 512], F32, bufs=1)
                            nc.scalar.activation(G[:], tmpf[:], AF.Sigmoid,
                                                 scale=-rwsf)
                            tmpi = pw.tile([128, 512], F32)
                            nc.vector.tensor_tensor(tmpi[:], psi[:],
                                                    S[:, t0:t0 + 512], OP.mult)
                            sgi = pw.tile([128, 512], F32, bufs=1)
                            nc.scalar.activation(sgi[:], tmpi[:], AF.Sigmoid,
                                                 scale=rwsi)
                            nc.vector.scalar_tensor_tensor(tmpi[:], tmpi[:],
                                                           rwsi, sgi[:],
                                                           OP.mult, OP.mult)
                            nc.vector.tensor_tensor(tmpi[:], tmpi[:], G[:],
                                                    OP.mult)
                            if half == 0:
                                hs[m] = hp.tile([128, TPC], F32, name=f"h_{m}")
                                fcs[m] = fcp.tile([128, TPC], FP16,
                                                  name=f"fc_{m}")
                                nc.vector.tensor_tensor_scan(
                                    hs[m][:, 0:512], F[:], tmpi[:], 0.0,
                                    OP.mult, OP.add)
                                nc.vector.tensor_tensor_scan(
                                    fcs[m][:, 0:512], F[:], zeros[:, 0:512],
                                    1.0, OP.mult, OP.add)
                            else:
                                nc.vector.tensor_tensor_scan(
                                    hs[m][:, 512:TPC], F[:], tmpi[:],
                                    hs[m][:, 511:512], OP.mult, OP.add)
                                nc.vector.tensor_tensor_scan(
                                    fcs[m][:, 512:TPC], F[:], zeros[:, 0:512],
                                    fcs[m][:, 511:512], OP.mult, OP.add)
                                nc.vector.tensor_copy(bnd[:, m:m + 1],
                                                      hs[m][:, TPC - 1:TPC])
                                # early chunked carry exchange
                                if m % 4 == 3:
                                    c = m // 4
                                    c0 = c * 4
                                    nc.vector.tensor_scalar_mul(
                                        bnd2[:, c0:c0 + 4], bnd[:, c0:c0 + 4],
                                        me[:])
                                    cin[c] = dram.tile([128, 4], F32,
                                                       name=f"cin_{c}")
                                    cout[c] = dram.tile([128, 4], F32,
                                                        name=f"cout_{c}")
                                    nc.sync.dma_start(cin[c][:],
                                                      bnd2[:, c0:c0 + 4])
                                    nc.gpsimd.collective_compute(
                                        "AllReduce", OP.add,
                                        replica_groups=[[0, 1], [2, 3],
                                                        [4, 5], [6, 7]],
                                        ins=[cin[c].opt()],
                                        outs=[cout[c].opt()],
                                    )
                            if half == 1 and m >= MT - 2:
                                load_g(m - (MT - 2))

                # ====== Phase TG: g-projection + gate (feature-major) =====
                # reuses the wi_m weight slots and psi/psf PSUM slots
                with tc.tile_pool(name="gw2", bufs=2) as gw2:
                    ps_ssg = psp.tile([1, TPC], F32, name="psf")
                    ps_ssp = psp.tile([1, TPC], F32, name="psf")
                    mxa = gw2.tile([128, TPC], F32, name="mxa", bufs=1)
                    nc.vector.memset(mxa[:], 0.0)
                    mxr = hp.tile([128, TPC], F32, name="mxr")
                    g2s = [None] * MT
                    o2s = [None] * MT

                    def issue_ssq(m):
                        for h in range(2):
                            nc.tensor.matmul(ps_ssg[:, h * 512:(h + 1) * 512],
                                             onescol[:],
                                             g2s[m][:, h * 512:(h + 1) * 512],
                                             start=(m == 0), stop=(m == MT - 1))
                            nc.tensor.matmul(ps_ssp[:, h * 512:(h + 1) * 512],
                                             onescol[:],
                                             o2s[m][:, h * 512:(h + 1) * 512],
                                             start=(m == 0), stop=(m == MT - 1))

                    for m in range(MT):
                        load_g(m + 2)
                        wg_m = gtiles.pop(m)
                        if m % 4 == 0:
                            # lazy read-back of carry chunk m//4
                            c = m // 4
                            c0 = c * 4
                            csb = gw2.tile([128, 4], F32, name=f"csb_{c}",
                                           bufs=1)
                            nc.sync.dma_start(csb[:], cout[c][:])
                            nc.vector.tensor_scalar_mul(
                                carried[:, c0:c0 + 4], csb[:], mo[:])
                        psg = psp.tile([128, TPC], F32, name="psi")
                        for k in range(KT):
                            lg = wg_m[:, k * 128:(k + 1) * 128]
                            st, sp = (k == 0), (k == KT - 1)
                            nc.tensor.matmul(psg[:, 0:512], lg,
                                             xqT3[:, k, 0:512], start=st, stop=sp)
                            nc.tensor.matmul(psg[:, 512:TPC], lg,
                                             xqT3[:, k, 512:TPC], start=st, stop=sp)
                        # gv = (psg*rwsg)*S
                        gvt = gw2.tile([128, TPC], F32, name="gvt")
                        nc.vector.scalar_tensor_tensor(gvt[:], psg[:], rwsg, S[:],
                                                       OP.mult, OP.mult)
                        g2s[m] = gw2.tile([128, TPC], BF16, name="g2")
                        nc.scalar.activation(g2s[m][:], gvt[:], AF.Square)
                        # carry fixup: h += fc * carry
                        nc.vector.scalar_tensor_tensor(
                            hs[m][:], fcs[m][:], carried[:, m:m + 1], hs[m][:],
                            OP.mult, OP.add)
                        hsg = gw2.tile([128, TPC], F32, name="hsg")
                        nc.scalar.activation(hsg[:], hs[m][:], AF.Sigmoid)
                        nc.vector.tensor_tensor(hs[m][:], hs[m][:], hsg[:],
                                                OP.mult)
                        # o_partial = (gv*gw_m) * (h*sig(h)), in place over hs
                        nc.vector.scalar_tensor_tensor(hs[m][:], gvt[:],
                                                       gwf[:, m:m + 1], hs[m][:],
                                                       OP.mult, OP.mult)
                        o2s[m] = gw2.tile([128, TPC], BF16, name="o2")
                        nc.scalar.activation(o2s[m][:], hs[m][:], AF.Square)
                        habs = gw2.tile([128, TPC], F32, name="hsg")
                        nc.scalar.activation(habs[:], hs[m][:], AF.Abs)
                        nc.vector.tensor_tensor(mxa[:], mxa[:], habs[:], OP.max)
                        if m >= 1:
                            issue_ssq(m - 1)
                    issue_ssq(MT - 1)

                    # ---- SC1 critical chain ----
                    nc.gpsimd.partition_all_reduce(mxr[:], mxa[:], 128,
                                                   bass_isa.ReduceOp.absmax)
                    nc.vector.tensor_scalar_max(rC[:], mxr[0:1, :], 1e-20)
                    nc.vector.reciprocal_approx_fast(out=srow[:], in_=rC[:])
                    nc.vector.tensor_scalar_mul(srow[:], srow[:], 127.0)
                    SC1r = srow
                    # stash the ssq rows to SBUF before the PSUM pool closes
                    for h in range(2):
                        nc.scalar.copy(rA[:, h * 512:(h + 1) * 512],
                                       ps_ssg[:, h * 512:(h + 1) * 512])
                        nc.scalar.copy(rB[:, h * 512:(h + 1) * 512],
                                       ps_ssp[:, h * 512:(h + 1) * 512])

            fcp_ctx.__exit__(None, None, None)
            xqp_ctx.__exit__(None, None, None)

            # ====== Phase TO: quantize o (feature-major) + out projection ====
            with (
                tc.tile_pool(name="oqp", bufs=1) as oqp,
                tc.tile_pool(name="wop", bufs=2) as wop,
                tc.tile_pool(name="ow", bufs=2) as ow,
            ):
                SC1b = ow.tile([128, TPC], F32, name="SC1b", bufs=1)
                nc.gpsimd.partition_broadcast(SC1b[:], SC1r[:])

                I16 = mybir.dt.int16
                oqT = oqp.tile([128, MT * TPC], BF16)
                oqT3 = oqT[:].rearrange("p (m t) -> p m t", m=MT)
                for m in range(MT):
                    oqi = ow.tile([128, TPC], I16, name="oqi")
                    nc.vector.tensor_tensor(oqi[:], hs[m][:], SC1b[:], OP.mult)
                    nc.vector.tensor_copy(oqT3[:, m, :], oqi[:])

                # deferred de-scale row math (runs during the o-projection):
                # rg = 1/sqrt(ssg/D+eps); ro = 1/sqrt(rg^2*ssp/D+eps)
                nc.scalar.activation(rC[:], rA[:], AF.Sqrt,
                                     bias=epsb[0:1, :], scale=1.0 / D)
                nc.vector.reciprocal_approx_fast(out=rA[:], in_=rC[:])
                nc.vector.tensor_tensor(rC[:], rA[:], rA[:], OP.mult)
                nc.vector.tensor_tensor(rC[:], rC[:], rB[:], OP.mult)
                nc.scalar.activation(rB[:], rC[:], AF.Sqrt,
                                     bias=epsb[0:1, :], scale=1.0 / D)
                nc.vector.reciprocal_approx_fast(out=rC[:], in_=rB[:])
                # SC2 = clip(mxp*rg*ro, eps)*rwso/127
                nc.vector.tensor_tensor(rA[:], rA[:], rC[:], OP.mult)
                nc.vector.tensor_tensor(rB[:], mxr[0:1, :], rA[:], OP.mult)
                nc.vector.tensor_scalar_max(rB[:], rB[:], EPS)
                nc.vector.tensor_scalar_mul(rB[:], rB[:], rwso / 127.0)
                sc2d = dram.tile([1, TPC], F32)
                nc.sync.dma_start(sc2d[:], rB[:])
                nc.sync.dma_start(
                    sc2col[:], sc2d[:].rearrange("o (t p) -> (o p) t", p=128))

                # o-projection, f-outer so PE consumes oq tiles as the
                # quantizer produces them (foq 0), then runs at full rate
                with tc.tile_pool(name="pso", bufs=8,
                                  space="PSUM") as pso_pool:
                    for foq in range(4):
                        wo = wop.tile([128, KT * 512], BF16)
                        for f in range(KT):
                            nc.sync.dma_start(wo[:, f * 512:(f + 1) * 512],
                                               wot_d.ap()[foq, f])
                        psos = [pso_pool.tile([128, 512], F32, name="pso")
                                for _ in range(NTT)]
                        for f in range(KT):
                            st, sp = (f == 0), (f == KT - 1)
                            for tti in range(NTT):
                                lo = oqT3[:, f, tti * 128:(tti + 1) * 128]
                                nc.tensor.matmul(psos[tti][:], lo,
                                                 wo[:, f * 512:(f + 1) * 512],
                                                 start=st, stop=sp)
                        for tti in range(NTT):
                            outsb = ow.tile([128, 512], F32, name="outsb")
                            nc.scalar.mul(outsb[:], psos[tti][:],
                                          sc2col[:, tti:tti + 1])
                            nc.sync.dma_start(
                                out_d.ap()[tti * 128:(tti + 1) * 128,
                                           foq * 512:(foq + 1) * 512],
                                outsb[:])

    nc.compile()
    return nc


_NC_CACHE = None
LAST_RESULTS = None


def _quant_weight(w):
    """fla BitLinear ternary weight quant. w [out, in] f32.
    Returns integer-valued f32 WT [in, out] and the reciprocal scale 1/ws."""
    import jax
    import jax.numpy as jnp

    mean_abs = np.asarray(
        jax.jit(lambda a: jnp.mean(jnp.abs(a)), backend="cpu")(w)
    )
    ws = np.float32(1.0) / np.maximum(mean_abs.astype(np.float32),
                                      np.float32(1e-5))
    wq = np.clip(np.round(w * ws), -1.0, 1.0).astype(np.float32)
    return wq.T.copy(), np.float32(1.0) / ws


def kernel(hidden_states, Wi, Wf, Wg, Wo, g_norm_weight):
    global _NC_CACHE, LAST_RESULTS

    wiq, rwsi = _quant_weight(np.asarray(Wi))
    wfq, rwsf = _quant_weight(np.asarray(Wf))
    wgq, rwsg = _quant_weight(np.asarray(Wg))
    woq, rwso = _quant_weight(np.asarray(Wo))

    if _NC_CACHE is None:
        _NC_CACHE = build_nc(float(rwsi), float(rwsf), float(rwsg),
                             float(rwso))
    nc = _NC_CACHE

    # [m][p][k][c] = WT[k*128+p, m*128+c]
    def tile_mk(wt):
        return np.ascontiguousarray(
            wt.reshape(KT, 128, MT, 128).transpose(2, 1, 0, 3)
        ).astype(ml_dtypes.bfloat16)

    wit = tile_mk(wiq)
    wft = tile_mk(wfq)
    wgt = tile_mk(wgq)
    # [foq][f][p][c] = WoT[f*128+p, foq*512+c]
    wot = np.ascontiguousarray(
        woq.reshape(KT, 128, 4, 512).transpose(2, 0, 1, 3)
    ).astype(ml_dtypes.bfloat16)

    gwf = np.ascontiguousarray(
        np.asarray(g_norm_weight, dtype=np.float32).reshape(MT, 128).T)
    x = np.asarray(hidden_states, dtype=np.float32)

    in_maps = []
    for c in range(NCORES):
        b, half = c // 2, c % 2
        in_maps.append({
            "x": np.ascontiguousarray(x[b, half * TPC:(half + 1) * TPC, :]),
            "wit": wit, "wft": wft, "wgt": wgt, "wot": wot,
            "gwf": gwf,
            "mask_even": np.full((128, 1), 1.0 - half, np.float32),
            "mask_odd": np.full((128, 1), float(half), np.float32),
        })

    import os
    trace = bool(os.environ.get("HGRN_TRACE"))
    res = run_bass_kernel_spmd(nc, in_maps, list(range(NCORES)), trace=trace)
    LAST_RESULTS = res
    out = np.empty((B, L, D), np.float32)
    for c in range(NCORES):
        b, half = c // 2, c % 2
        out[b, half * TPC:(half + 1) * TPC, :] = res.results[c]["out"]
    return out


# revision 47
# speedup vs baseline: 1.0154x; 1.0154x over previous
"""HGRN BitAttention Trainium2 kernel (8-core SPMD, token-sharded).

Sharding: core c handles batch c//2, sequence half c%2 (1024 tokens).
The HGRN recurrence carry crosses the half boundary via small pair
AllReduces (4 chunks, issued early so the latency hides under the
g-projection); masks make the program uniform (SPMD).

BitLinear trick: activations quantize to integers in [-127,127] and
weights to {-1,0,1} - both exact in bf16 - so all four projections are
exact-integer bf16 matmuls with fp32 PSUM accumulation; per-token /
per-weight scales are applied outside the matmuls.

Layout: everything except the final output projection result is
feature-major [feature, token].  The gate chain is algebraically
reduced so that per-token normalizers cancel before rounding:
  o_partial = g*(1/s_x)(1/ws_g)*gw * h*sigmoid(h)
  oq        = round(o_partial * 127/max_f|o_partial|)
  out       = (oq @ WoT) * SC2,  SC2 = rstd_o*rstd_g*mxp*rwso/127
Per-token sumsq reductions run on the PE (ones-column matmul), the
per-token max runs as a DVE partition-halving tree.  The o-projection
takes feature-major oq tiles as the stationary operand, so its output
is token-major with no explicit transposes anywhere except the initial
x quantization (DMA xbar).
"""

import numpy as np
import ml_dtypes

import concourse.bass as bass
import concourse.bass_isa as bass_isa
import concourse.bacc as bacc
import concourse.mybir as mybir
import concourse.tile as tile
from concourse.bass_utils import run_bass_kernel_spmd

F32 = mybir.dt.float32
BF16 = mybir.dt.bfloat16
FP16 = mybir.dt.float16
I32 = mybir.dt.int32
AF = mybir.ActivationFunctionType
OP = mybir.AluOpType

B, L, D = 4, 2048, 2048
NCORES = 8
TPC = L // 2          # tokens per core = 1024
NTT = TPC // 128      # 8 token tiles per core
KT = D // 128         # 16 k tiles
MT = D // 128         # 16 m tiles
CCH = 4               # carry-exchange chunks (4 m-tiles each)
EPS = 1e-5


def build_nc(rwsi, rwsf, rwsg, rwso):
    nc = bacc.Bacc("TRN2", target_bir_lowering=False, debug=False,
                   num_devices=NCORES)

    x_d = nc.dram_tensor("x", [TPC, D], F32, kind="ExternalInput")
    wit_d = nc.dram_tensor("wit", [MT, 128, KT, 128], BF16, kind="ExternalInput")
    wft_d = nc.dram_tensor("wft", [MT, 128, KT, 128], BF16, kind="ExternalInput")
    wgt_d = nc.dram_tensor("wgt", [MT, 128, KT, 128], BF16, kind="ExternalInput")
    wot_d = nc.dram_tensor("wot", [4, KT, 128, 512], BF16, kind="ExternalInput")
    gwf_d = nc.dram_tensor("gwf", [128, MT], F32, kind="ExternalInput")
    me_d = nc.dram_tensor("mask_even", [128, 1], F32, kind="ExternalInput")
    mo_d = nc.dram_tensor("mask_odd", [128, 1], F32, kind="ExternalInput")
    out_d = nc.dram_tensor("out", [TPC, D], F32, kind="ExternalOutput")

    with tile.TileContext(nc) as tc:
        with (
            tc.tile_pool(name="const", bufs=1) as cp,
            tc.tile_pool(name="hp", bufs=1) as hp,
            tc.tile_pool(name="dram", bufs=1, space="DRAM") as dram,
        ):
            # ---- constants ----
            me = cp.tile([128, 1], F32)
            nc.sync.dma_start(me[:], me_d.ap())
            mo = cp.tile([128, 1], F32)
            nc.sync.dma_start(mo[:], mo_d.ap())
            gwf = cp.tile([128, MT], F32)
            nc.sync.dma_start(gwf[:], gwf_d.ap())
            epsb = cp.tile([128, 1], F32)
            nc.vector.memset(epsb[:], EPS)
            zeros = cp.tile([128, TPC], F32)
            nc.vector.memset(zeros[:], 0.0)
            ones1 = cp.tile([1, 128], F32)
            nc.vector.memset(ones1[:], 1.0)
            onescol = cp.tile([128, 1], BF16)
            nc.vector.memset(onescol[:], 1.0)

            srec = [cp.tile([128, 4], F32, name=f"srec_{h}")
                    for h in range(2)]          # (1/s_x) per token, per half
            Sh = [cp.tile([128, 512], F32, name=f"S_{h}")
                  for h in range(2)]            # (1/s_x) feature-major bcast
            bnd = cp.tile([128, MT], F32)
            bnd2 = cp.tile([128, MT], F32)
            carried = cp.tile([128, MT], F32)
            sc2col = cp.tile([128, NTT], F32)
            srow = [cp.tile([1, 512], F32, name=f"srow_{h}")
                    for h in range(2)]
            rA = cp.tile([1, TPC], F32)
            rB = cp.tile([1, TPC], F32)
            rC = cp.tile([1, TPC], F32)

            hs = [None] * MT

            xqp_ctx = tc.tile_pool(name="xq", bufs=1)
            xqp = xqp_ctx.__enter__()
            xqT = xqp.tile([128, KT * TPC], BF16)
            xqT3 = xqT[:].rearrange("p (k t) -> p k t", k=KT)

            fcp_ctx = tc.tile_pool(name="fcp", bufs=1)
            fcp = fcp_ctx.__enter__()
            fcs = [None] * MT

            # ================= Phase X: quantize x (token-major stats) ========
            with (
                tc.tile_pool(name="xin", bufs=3) as xin,
                tc.tile_pool(name="xw", bufs=2) as xw,
            ):
                xts = {}

                def load_x(tt):
                    if tt >= NTT:
                        return
                    xt = xin.tile([128, D], F32, name="xt")
                    nc.sync.dma_start(xt[:],
                                      x_d.ap()[tt * 128:(tt + 1) * 128, :])
                    xts[tt] = xt

                qbs = {}

                def transpose(tt):
                    if tt < 0:
                        return
                    nc.scalar.dma_start_transpose(
                        xqT3[:, :, tt * 128:(tt + 1) * 128], qbs.pop(tt)[:])

                load_x(0)
                load_x(1)
                for tt in range(NTT):
                    load_x(tt + 2)
                    transpose(tt - 1)
                    xt = xts.pop(tt)
                    scr = xw.tile([128, D], F32, bufs=1)
                    ssum = xw.tile([128, 1], F32)
                    nc.scalar.activation(scr[:], xt[:], AF.Square,
                                         accum_out=ssum[:])
                    std = xw.tile([128, 1], F32)
                    nc.scalar.activation(std[:], ssum[:], AF.Sqrt,
                                         bias=epsb[:], scale=1.0 / D)
                    rstd = xw.tile([128, 1], F32)
                    nc.vector.reciprocal(rstd[:], std[:])
                    mx = xw.tile([128, 1], F32)
                    nc.vector.tensor_reduce(mx[:], xt[:], mybir.AxisListType.X,
                                            OP.max, apply_absolute_value=True)
                    a = xw.tile([128, 1], F32)
                    nc.vector.tensor_tensor(a[:], rstd[:], mx[:], OP.mult)
                    nc.vector.tensor_scalar_max(a[:], a[:], EPS)
                    nc.vector.tensor_scalar_mul(
                        srec[tt // 4][:, tt % 4:tt % 4 + 1], a[:], 1.0 / 127.0)
                    r1 = xw.tile([128, 1], F32)
                    nc.vector.reciprocal(r1[:], a[:])
                    qsc = xw.tile([128, 1], F32)
                    nc.vector.tensor_tensor(qsc[:], r1[:], rstd[:], OP.mult)
                    nc.vector.tensor_scalar_mul(qsc[:], qsc[:], 127.0)
                    qi = xw.tile([128, D], I32, bufs=1)
                    nc.vector.tensor_scalar_mul(qi[:], xt[:], qsc[:])
                    qb = xw.tile([128, D], BF16)
                    nc.scalar.copy(qb[:], qi[:])
                    qbs[tt] = qb

                    # stage + broadcast the per-token scale for this half
                    if tt % 4 == 3:
                        c = tt // 4
                        srd = dram.tile([1, 512], F32, name=f"srd_{c}")
                        nc.sync.dma_start(
                            srd[:].rearrange("o (t p) -> (o p) t", p=128),
                            srec[c][:])
                        nc.sync.dma_start(srow[c][:], srd[:])
                        nc.gpsimd.partition_broadcast(Sh[c][:], srow[c][:])
                transpose(NTT - 1)

            # ====== Phase P: i/f projections + scans (feature-major) ======
            cin = [None] * CCH
            cout = [None] * CCH
            with (
                tc.tile_pool(name="wif", bufs=2) as wif,
                tc.tile_pool(name="psp", bufs=2, space="PSUM") as psp,
            ):
                # ====== Phase P: i/f projections + scans (feature-major) =====
                # two passes over token halves; pass 1 starts as soon as the
                # first half of x is quantized; scans chain across the halves
                wtiles = {}
                gtiles = {}

                def load_if(j):
                    if j >= 2 * MT:
                        return
                    # pass-1 weights ride the gpsimd queue (idle during the
                    # x-quantization); later ones the sync queue (drained)
                    eng = nc.gpsimd if j <= MT else nc.sync
                    mm = j % MT
                    wi = wif.tile([128, KT * 128], BF16, name="wi_m")
                    eng.dma_start(
                        wi[:], wit_d.ap()[mm].rearrange("p k c -> p (k c)"))
                    wf = wif.tile([128, KT * 128], BF16, name="wf_m")
                    eng.dma_start(
                        wf[:], wft_d.ap()[mm].rearrange("p k c -> p (k c)"))
                    wtiles[j] = (wi, wf)

                def load_g(mm):
                    if mm >= MT:
                        return
                    wg = wif.tile([128, KT * 128], BF16, name="wf_m")
                    nc.sync.dma_start(
                        wg[:], wgt_d.ap()[mm].rearrange("p k c -> p (k c)"))
                    gtiles[mm] = wg

                with tc.tile_pool(name="pw", bufs=2) as pw:
                    load_if(0)
                    for half in range(2):
                        t0 = half * 512
                        for m in range(MT):
                            j = half * MT + m
                            load_if(j + 1)
                            wi_m, wf_m = wtiles.pop(j)
                            psi = psp.tile([128, 512], F32, name="psi")
                            psf = psp.tile([128, 512], F32, name="psf")
                            for k in range(KT):
                                li = wi_m[:, k * 128:(k + 1) * 128]
                                lf = wf_m[:, k * 128:(k + 1) * 128]
                                st, sp = (k == 0), (k == KT - 1)
                                nc.tensor.matmul(psi[:], li,
                                                 xqT3[:, k, t0:t0 + 512],
                                                 start=st, stop=sp)
                                nc.tensor.matmul(psf[:], lf,
                                                 xqT3[:, k, t0:t0 + 512],
                                                 start=st, stop=sp)
                            tmpf = pw.tile([128, 512], F32, bufs=1)
                            nc.vector.tensor_tensor(tmpf[:], psf[:],
                                                    Sh[half][:], OP.mult)
                            F = pw.tile([128, 512], F32, bufs=1)
                            nc.scalar.activation(F[:], tmpf[:], AF.Sigmoid,
                                                 scale=rwsf)
                            G = pw.tile([128, 512], F32, bufs=1)
                            nc.scalar.activation(G[:], tmpf[:], AF.Sigmoid,
                                                 scale=-rwsf)
                            tmpi = pw.tile([128, 512], F32)
                            nc.vector.tensor_tensor(tmpi[:], psi[:],
                                                    Sh[half][:], OP.mult)
                            sgi = pw.tile([128, 512], F32, bufs=1)
                            nc.scalar.activation(sgi[:], tmpi[:], AF.Sigmoid,
                                                 scale=rwsi)
                            nc.vector.scalar_tensor_tensor(tmpi[:], tmpi[:],
                                                           rwsi, sgi[:],
                                                           OP.mult, OP.mult)
                            nc.vector.tensor_tensor(tmpi[:], tmpi[:], G[:],
                                                    OP.mult)
                            if half == 0:
                                hs[m] = hp.tile([128, TPC], F32, name=f"h_{m}")
                                fcs[m] = fcp.tile([128, TPC], FP16,
                                                  name=f"fc_{m}")
                                nc.vector.tensor_tensor_scan(
                                    hs[m][:, 0:512], F[:], tmpi[:], 0.0,
                                    OP.mult, OP.add)
                                nc.vector.tensor_tensor_scan(
                                    fcs[m][:, 0:512], F[:], zeros[:, 0:512],
                                    1.0, OP.mult, OP.add)
                            else:
                                nc.vector.tensor_tensor_scan(
                                    hs[m][:, 512:TPC], F[:], tmpi[:],
                                    hs[m][:, 511:512], OP.mult, OP.add)
                                nc.vector.tensor_tensor_scan(
                                    fcs[m][:, 512:TPC], F[:], zeros[:, 0:512],
                                    fcs[m][:, 511:512], OP.mult, OP.add)
                                nc.vector.tensor_copy(bnd[:, m:m + 1],
                                                      hs[m][:, TPC - 1:TPC])
                                # early chunked carry exchange
                                if m % 4 == 3:
                                    c = m // 4
                                    c0 = c * 4
                                    nc.vector.tensor_scalar_mul(
                                        bnd2[:, c0:c0 + 4], bnd[:, c0:c0 + 4],
                                        me[:])
                                    cin[c] = dram.tile([128, 4], F32,
                                                       name=f"cin_{c}")
                                    cout[c] = dram.tile([128, 4], F32,
                                                        name=f"cout_{c}")
                                    nc.sync.dma_start(cin[c][:],
                                                      bnd2[:, c0:c0 + 4])
                                    nc.gpsimd.collective_compute(
                                        "AllReduce", OP.add,
                                        replica_groups=[[0, 1], [2, 3],
                                                        [4, 5], [6, 7]],
                                        ins=[cin[c].opt()],
                                        outs=[cout[c].opt()],
                                    )
                            if half == 1 and m >= MT - 2:
                                load_g(m - (MT - 2))

                # ====== Phase TG: g-projection + gate (feature-major) =====
                # reuses the wi_m weight slots and psi/psf PSUM slots
                with tc.tile_pool(name="gw2", bufs=2) as gw2:
                    ps_ssg = psp.tile([1, TPC], F32, name="psf")
                    ps_ssp = psp.tile([1, TPC], F32, name="psf")
                    mxa = gw2.tile([128, TPC], F32, name="mxa", bufs=1)
                    nc.vector.memset(mxa[:], 0.0)
                    mxr = hp.tile([128, TPC], F32, name="mxr")
                    g2s = [None] * MT
                    o2s = [None] * MT

                    def issue_ssq(m):
                        for h in range(2):
                            nc.tensor.matmul(ps_ssg[:, h * 512:(h + 1) * 512],
                                             onescol[:],
                                             g2s[m][:, h * 512:(h + 1) * 512],
                                             start=(m == 0), stop=(m == MT - 1))
                            nc.tensor.matmul(ps_ssp[:, h * 512:(h + 1) * 512],
                                             onescol[:],
                                             o2s[m][:, h * 512:(h + 1) * 512],
                                             start=(m == 0), stop=(m == MT - 1))

                    for m in range(MT):
                        load_g(m + 2)
                        wg_m = gtiles.pop(m)
                        if m % 4 == 0:
                            # lazy read-back of carry chunk m//4
                            c = m // 4
                            c0 = c * 4
                            csb = gw2.tile([128, 4], F32, name=f"csb_{c}",
                                           bufs=1)
                            nc.sync.dma_start(csb[:], cout[c][:])
                            nc.vector.tensor_scalar_mul(
                                carried[:, c0:c0 + 4], csb[:], mo[:])
                        psg = psp.tile([128, TPC], F32, name="psi")
                        for k in range(KT):
                            lg = wg_m[:, k * 128:(k + 1) * 128]
                            st, sp = (k == 0), (k == KT - 1)
                            nc.tensor.matmul(psg[:, 0:512], lg,
                                             xqT3[:, k, 0:512], start=st, stop=sp)
                            nc.tensor.matmul(psg[:, 512:TPC], lg,
                                             xqT3[:, k, 512:TPC], start=st, stop=sp)
                        # gv = (psg*rwsg)*S
                        gvt = gw2.tile([128, TPC], F32, name="gvt")
                        nc.vector.scalar_tensor_tensor(
                            gvt[:, 0:512], psg[:, 0:512], rwsg, Sh[0][:],
                            OP.mult, OP.mult)
                        nc.vector.scalar_tensor_tensor(
                            gvt[:, 512:TPC], psg[:, 512:TPC], rwsg, Sh[1][:],
                            OP.mult, OP.mult)
                        g2s[m] = gw2.tile([128, TPC], BF16, name="g2")
                        nc.scalar.activation(g2s[m][:], gvt[:], AF.Square)
                        # carry fixup: h += fc * carry
                        nc.vector.scalar_tensor_tensor(
                            hs[m][:], fcs[m][:], carried[:, m:m + 1], hs[m][:],
                            OP.mult, OP.add)
                        hsg = gw2.tile([128, TPC], F32, name="hsg")
                        nc.scalar.activation(hsg[:], hs[m][:], AF.Sigmoid)
                        nc.vector.tensor_tensor(hs[m][:], hs[m][:], hsg[:],
                                                OP.mult)
                        # o_partial = (gv*gw_m) * (h*sig(h)), in place over hs
                        nc.vector.scalar_tensor_tensor(hs[m][:], gvt[:],
                                                       gwf[:, m:m + 1], hs[m][:],
                                                       OP.mult, OP.mult)
                        o2s[m] = gw2.tile([128, TPC], BF16, name="o2")
                        nc.scalar.activation(o2s[m][:], hs[m][:], AF.Square)
                        habs = gw2.tile([128, TPC], F32, name="hsg")
                        nc.scalar.activation(habs[:], hs[m][:], AF.Abs)
                        nc.vector.tensor_tensor(mxa[:], mxa[:], habs[:], OP.max)
                        if m >= 1:
                            issue_ssq(m - 1)
                    issue_ssq(MT - 1)

                    # ---- SC1 critical chain ----
                    nc.gpsimd.partition_all_reduce(mxr[:], mxa[:], 128,
                                                   bass_isa.ReduceOp.absmax)
                    nc.vector.tensor_scalar_max(rC[:], mxr[0:1, :], 1e-20)
                    SC1r = zeros[0:1, :]    # fc-scan zeros are dead now
                    nc.vector.reciprocal_approx_fast(out=SC1r, in_=rC[:])
                    nc.vector.tensor_scalar_mul(SC1r, SC1r, 127.0)
                    # stash the ssq rows to SBUF before the PSUM pool closes
                    for h in range(2):
                        nc.scalar.copy(rA[:, h * 512:(h + 1) * 512],
                                       ps_ssg[:, h * 512:(h + 1) * 512])
                        nc.scalar.copy(rB[:, h * 512:(h + 1) * 512],
                                       ps_ssp[:, h * 512:(h + 1) * 512])

            fcp_ctx.__exit__(None, None, None)
            xqp_ctx.__exit__(None, None, None)

            # ====== Phase TO: quantize o (feature-major) + out projection ====
            with (
                tc.tile_pool(name="oqp", bufs=1) as oqp,
                tc.tile_pool(name="wop", bufs=2) as wop,
                tc.tile_pool(name="ow", bufs=2) as ow,
            ):
                SC1b = ow.tile([128, TPC], F32, name="SC1b", bufs=1)
                nc.gpsimd.partition_broadcast(SC1b[:], SC1r)

                I16 = mybir.dt.int16
                oqT = oqp.tile([128, MT * TPC], BF16)
                oqT3 = oqT[:].rearrange("p (m t) -> p m t", m=MT)
                for m in range(MT):
                    oqi = ow.tile([128, TPC], I16, name="oqi")
                    nc.vector.tensor_tensor(oqi[:], hs[m][:], SC1b[:], OP.mult)
                    nc.vector.tensor_copy(oqT3[:, m, :], oqi[:])

                # deferred de-scale row math (runs during the o-projection):
                # rg = 1/sqrt(ssg/D+eps); ro = 1/sqrt(rg^2*ssp/D+eps)
                nc.scalar.activation(rC[:], rA[:], AF.Sqrt,
                                     bias=epsb[0:1, :], scale=1.0 / D)
                nc.vector.reciprocal_approx_fast(out=rA[:], in_=rC[:])
                nc.vector.tensor_tensor(rC[:], rA[:], rA[:], OP.mult)
                nc.vector.tensor_tensor(rC[:], rC[:], rB[:], OP.mult)
                nc.scalar.activation(rB[:], rC[:], AF.Sqrt,
                                     bias=epsb[0:1, :], scale=1.0 / D)
                nc.vector.reciprocal_approx_fast(out=rC[:], in_=rB[:])
                # SC2 = clip(mxp*rg*ro, eps)*rwso/127
                nc.vector.tensor_tensor(rA[:], rA[:], rC[:], OP.mult)
                nc.vector.tensor_tensor(rB[:], mxr[0:1, :], rA[:], OP.mult)
                nc.vector.tensor_scalar_max(rB[:], rB[:], EPS)
                nc.vector.tensor_scalar_mul(rB[:], rB[:], rwso / 127.0)
                sc2d = dram.tile([1, TPC], F32)
                nc.sync.dma_start(sc2d[:], rB[:])
                nc.sync.dma_start(
                    sc2col[:], sc2d[:].rearrange("o (t p) -> (o p) t", p=128))

                # o-projection, f-outer so PE consumes oq tiles as the
                # quantizer produces them (foq 0), then runs at full rate
                with tc.tile_pool(name="pso", bufs=8,
                                  space="PSUM") as pso_pool:
                    for foq in range(4):
                        wo = wop.tile([128, KT * 512], BF16)
                        for f in range(KT):
                            nc.sync.dma_start(wo[:, f * 512:(f + 1) * 512],
                                               wot_d.ap()[foq, f])
                        psos = [pso_pool.tile([128, 512], F32, name="pso")
                                for _ in range(NTT)]
                        for f in range(KT):
                            st, sp = (f == 0), (f == KT - 1)
                            for tti in range(NTT):
                                lo = oqT3[:, f, tti * 128:(tti + 1) * 128]
                                nc.tensor.matmul(psos[tti][:], lo,
                                                 wo[:, f * 512:(f + 1) * 512],
                                                 start=st, stop=sp)
                        for tti in range(NTT):
                            outsb = ow.tile([128, 512], F32, name="outsb")
                            nc.scalar.mul(outsb[:], psos[tti][:],
                                          sc2col[:, tti:tti + 1])
                            nc.sync.dma_start(
                                out_d.ap()[tti * 128:(tti + 1) * 128,
                                           foq * 512:(foq + 1) * 512],
                                outsb[:])

    nc.compile()
    return nc


_NC_CACHE = None
LAST_RESULTS = None


def _quant_weight(w):
    """fla BitLinear ternary weight quant. w [out, in] f32.
    Returns integer-valued f32 WT [in, out] and the reciprocal scale 1/ws."""
    import jax
    import jax.numpy as jnp

    mean_abs = np.asarray(
        jax.jit(lambda a: jnp.mean(jnp.abs(a)), backend="cpu")(w)
    )
    ws = np.float32(1.0) / np.maximum(mean_abs.astype(np.float32),
                                      np.float32(1e-5))
    wq = np.clip(np.round(w * ws), -1.0, 1.0).astype(np.float32)
    return wq.T.copy(), np.float32(1.0) / ws


def kernel(hidden_states, Wi, Wf, Wg, Wo, g_norm_weight):
    global _NC_CACHE, LAST_RESULTS

    wiq, rwsi = _quant_weight(np.asarray(Wi))
    wfq, rwsf = _quant_weight(np.asarray(Wf))
    wgq, rwsg = _quant_weight(np.asarray(Wg))
    woq, rwso = _quant_weight(np.asarray(Wo))

    if _NC_CACHE is None:
        _NC_CACHE = build_nc(float(rwsi), float(rwsf), float(rwsg),
                             float(rwso))
    nc = _NC_CACHE

    # [m][p][k][c] = WT[k*128+p, m*128+c]
    def tile_mk(wt):
        return np.ascontiguousarray(
            wt.reshape(KT, 128, MT, 128).transpose(2, 1, 0, 3)
        ).astype(ml_dtypes.bfloat16)

    wit = tile_mk(wiq)
    wft = tile_mk(wfq)
    wgt = tile_mk(wgq)
    # [foq][f][p][c] = WoT[f*128+p, foq*512+c]
    wot = np.ascontiguousarray(
        woq.reshape(KT, 128, 4, 512).transpose(2, 0, 1, 3)
    ).astype(ml_dtypes.bfloat16)

    gwf = np.ascontiguousarray(
        np.asarray(g_norm_weight, dtype=np.float32).reshape(MT, 128).T)
    x = np.asarray(hidden_states, dtype=np.float32)

    in_maps = []
    for c in range(NCORES):
        b, half = c // 2, c % 2
        in_maps.append({
            "x": np.ascontiguousarray(x[b, half * TPC:(half + 1) * TPC, :]),
            "wit": wit, "wft": wft, "wgt": wgt, "wot": wot,
            "gwf": gwf,
            "mask_even": np.full((128, 1), 1.0 - half, np.float32),
            "mask_odd": np.full((128, 1), float(half), np.float32),
        })

    import os
    trace = bool(os.environ.get("HGRN_TRACE"))
    res = run_bass_kernel_spmd(nc, in_maps, list(range(NCORES)), trace=trace)
    LAST_RESULTS = res
    out = np.empty((B, L, D), np.float32)
    for c in range(NCORES):
        b, half = c // 2, c % 2
        out[b, half * TPC:(half + 1) * TPC, :] = res.results[c]["out"]
    return out
